# revision 7
# baseline (speedup 1.0000x reference)
"""GNN message-passing (scatter-add) kernel for 8 Trainium2 NeuronCores.

Computes out = segment_sum(x[src], dst, num_segments=N) for
x [10000, 128] f32, edge_index [2, 320000] int64.

Strategy — BIT-PLANE dense count-matrix matmul (no per-edge work):
  out^T[f, d] = sum_s A[s, d] * x[s, f]   with A[s, d] = #edges s->d.

  A is ~0/1 (density 0.3%). Store min(A, 1) as 5 bit-planes packed one
  byte per 5 dst columns (bit pl of byte[s, j] = A[s, 256*pl + j] >= 1):
  the A stream shrinks from 12.8MB dense fp8 to 2.6MB per core.

  On device the DVE expands each plane with one bitwise-AND
  (mask 0x0101<<pl on a uint16 view, two fp8 lanes per elem). The
  masked bytes ARE valid fp8e4m3: a lone bit pl is the exact power of
  two v_pl in [2^-9 .. 2^-5], so the matmul accumulates v_pl * partial
  and the PSUM drain rescales by the exact inverse.

  Cells with count >= 2 (~60 per core) are patched by one extra
  contraction chunk: their x rows are fetched on-device by an indirect
  (indexed) DMA, and a host-built residual matrix rmat (= (count-1) *
  v_pl at the patched cells) joins the same PSUM accumulation.

  dst is sharded across the 8 cores (core c owns cols [c*1264, +1264));
  no collectives. x rides fp16; out written fp16 (rel err ~3e-4).

  Schedule: matmuls start chunk-major while the DMA stream ramps
  (3 psum banks fed per chunk), then switch bank-major so each PSUM
  bank's drain + store overlaps the next bank's matmuls; only the last
  bank's drain is exposed. LDWEIGHTS fully overlaps the matmul stream
  (verified on traces), so the kernel runs at the PE column rate:
  79*1264 fp8 columns ~= 43us, plus ~5us preamble and ~3us tail.
"""

import sys

for _p in ("/opt/trn_rl_repo",):
    if _p not in sys.path:
        sys.path.append(_p)


def _install_axon_ntff_hook_shim():
    # Some images ship an antenv without axon_hooks; bass_utils then
    # crashes on trace=True under axon. Provide the module and register
    # the ctypes NTFF hook the same way trn_boot would. Fully guarded —
    # a no-op wherever the real module exists.
    import types

    try:
        import antenv.axon_hooks  # noqa: F401

        return
    except ImportError:
        pass
    try:
        import antenv

        mod = types.ModuleType("antenv.axon_hooks")
        mod._hook = None

        def set_axon_ntff_profile_hook(h):
            mod._hook = h

        def get_axon_ntff_profile_hook():
            return mod._hook

        mod.set_axon_ntff_profile_hook = set_axon_ntff_profile_hook
        mod.get_axon_ntff_profile_hook = get_axon_ntff_profile_hook
        sys.modules["antenv.axon_hooks"] = mod
        antenv.axon_hooks = mod
        from trn_agent_boot.trn_boot import _ntff_profile_via_ctypes

        mod._hook = _ntff_profile_via_ctypes("/opt/axon/libaxon_pjrt.so")
    except Exception:
        pass


_install_axon_ntff_hook_shim()

import ml_dtypes
import numpy as np

import concourse.bacc as bacc
import concourse.mybir as mybir
import concourse.tile as tile
from concourse import bass
from concourse.bass_utils import run_bass_kernel_spmd

N_NODES = 10000
D_FEAT = 128
N_CORES = 8
P = 128
KCH = -(-N_NODES // P)  # 79 source chunks
NPAD = KCH * P  # 10112
DCORE = NPAD // N_CORES  # 1264 dst columns per core
PLANES = 5
W = 256  # dst cols per plane; 5*256 = 1280 >= 1264 (last plane 240 used)
WU = W // 2  # uint16 lanes per plane
W_LAST = DCORE - 4 * W  # 240
# fp8e4m3 value of a lone bit pl (exact powers of two)
BITVAL = [2.0**-9, 2.0**-8, 2.0**-7, 2.0**-6, 2.0**-5]
DRAIN_SCALE = [512.0, 256.0, 128.0, 64.0, 32.0]
# psum bank t covers planes BANKPL[t] with total width BANKW[t]
BANKPL = [(0, 1), (2, 3), (4,)]
BANKW = [2 * W, 2 * W, W_LAST]
FP8 = ml_dtypes.float8_e4m3

# test/profiling hooks
TRACE = False
TRACE_CORES = None
LAST_RESULT = None


def _groups(sizes):
    out = []
    k0 = 0
    for g in sizes:
        out.append((k0, g))
        k0 += g
    assert k0 == KCH
    return out


# chunk-load groups; the first ones are small for a fast PE start
GSIZES = [4, 4, 8, 8, 8, 8, 8, 8, 8, 8, 7]
KGROUPS = _groups(GSIZES)
# chunk-major (all banks per chunk) while the DMA stream ramps, then
# bank-major so bank drains overlap the next bank's matmuls
PHASE_A_GROUPS = 5  # chunks 0..31
KSPLIT = sum(GSIZES[:PHASE_A_GROUPS])


def _build_program():
    nc = bacc.Bacc(
        "TRN2", target_bir_lowering=False, debug=False, num_devices=N_CORES
    )
    xt_d = nc.dram_tensor(
        "xt", [P, KCH * D_FEAT], mybir.dt.float16, kind="ExternalInput"
    )
    apk_d = nc.dram_tensor(
        "apk", [P, KCH * W], mybir.dt.uint8, kind="ExternalInput"
    )
    xr_d = nc.dram_tensor(
        "xr", [NPAD, D_FEAT], mybir.dt.float16, kind="ExternalInput"
    )
    pidx_d = nc.dram_tensor("pidx", [P, 1], mybir.dt.int32, kind="ExternalInput")
    rmat_d = nc.dram_tensor(
        "rmat", [P, PLANES * W], mybir.dt.float8e4, kind="ExternalInput"
    )
    o_d = nc.dram_tensor("o", [P, DCORE], mybir.dt.float16, kind="ExternalOutput")

    xv = xt_d[:].rearrange("p (k f) -> p k f", k=KCH, f=D_FEAT)
    av = apk_d[:].rearrange("p (k w) -> p k w", k=KCH, w=W)

    with tile.TileContext(nc) as tc:
        with (
            tc.tile_pool(name="xt", bufs=1) as xtp,
            tc.tile_pool(name="a", bufs=1) as ap_,
            tc.tile_pool(name="pl", bufs=2) as plp,
            tc.tile_pool(name="patch", bufs=1) as pp,
            tc.tile_pool(name="res", bufs=2) as resp,
            tc.tile_pool(name="ps", bufs=1, space="PSUM") as psp,
        ):
            # patch inputs ride the (otherwise idle) gpsimd queue
            pidx_sb = pp.tile([P, 1], mybir.dt.int32, name="pidx_sb")
            nc.gpsimd.dma_start(out=pidx_sb[:], in_=pidx_d[:])
            rmat_sb = pp.tile([P, PLANES, W], mybir.dt.float8e4, name="rmat_sb")
            nc.gpsimd.dma_start(
                out=rmat_sb[:],
                in_=rmat_d[:].rearrange("p (l w) -> p l w", l=PLANES, w=W),
            )
            xp_sb = pp.tile([P, D_FEAT], mybir.dt.float16, name="xp_sb")
            nc.gpsimd.indirect_dma_start(
                out=xp_sb[:],
                out_offset=None,
                in_=xr_d[:],
                in_offset=bass.IndirectOffsetOnAxis(ap=pidx_sb[:, :1], axis=0),
            )

            # xt and apk stay fully resident; loads stream group-wise on
            # two HWDGE queues. One persistent tile PER GROUP so deps are
            # group-granular regardless of the framework's AP tracking.
            xt_g = {}
            a16_g = {}
            for gi, (k0, gn) in enumerate(KGROUPS):
                xg = xtp.tile(
                    [P, gn, D_FEAT], mybir.dt.float16,
                    tag=f"xt{gi}", name=f"xt{gi}",
                )
                nc.scalar.dma_start(out=xg[:], in_=xv[:, k0 : k0 + gn, :])
                ag = ap_.tile(
                    [P, gn, W], mybir.dt.uint8, tag=f"a{gi}", name=f"a{gi}"
                )
                nc.sync.dma_start(out=ag[:], in_=av[:, k0 : k0 + gn, :])
                xt_g[k0] = xg
                a16_g[k0] = ag[:].bitcast(mybir.dt.uint16)  # [P, gn, WU]

            pss = [
                psp.tile(
                    [P, BANKW[t]], mybir.dt.float32, tag=f"ps{t}", name=f"ps{t}"
                )
                for t in range(3)
            ]

            def unpack(pls, k0, gn, gi):
                # one AND per plane over a chunk group; masked bytes are
                # read back as fp8 by the matmuls
                t_sb = plp.tile(
                    [P, gn, len(pls), WU],
                    mybir.dt.uint16,
                    tag=f"pl{len(pls)}_{gn}",
                    name=f"pl{pls[0]}_{gi}",
                )
                for i, pl in enumerate(pls):
                    nc.vector.tensor_scalar(
                        out=t_sb[:, :, i, :],
                        in0=a16_g[k0],
                        scalar1=(0x0101 << pl) & 0xFFFF,
                        scalar2=None,
                        op0=mybir.AluOpType.bitwise_and,
                    )
                return t_sb

            def bank_mm(t, kk, t_sb, i0, g0):
                # matmul for psum bank t, contraction chunk kk; planes of
                # bank t sit at index i0.. in t_sb (group base g0)
                npl = len(BANKPL[t])
                wid = BANKW[t] // npl // 2  # u16 lanes per plane used
                rhs = t_sb[:, kk - g0, i0 : i0 + npl, :wid].bitcast(
                    mybir.dt.float8e4
                )
                nc.tensor.matmul(
                    pss[t][:],
                    xt_g[g0][:, kk - g0, :],
                    rhs,
                    start=(kk == 0),
                    stop=False,
                )

            def drain(t, halves=1):
                w_all = BANKW[t]
                res = resp.tile(
                    [P, w_all], mybir.dt.float16, tag=f"res{t}", name=f"res{t}"
                )
                for i, pl in enumerate(BANKPL[t]):
                    wid = w_all // len(BANKPL[t])
                    nc.vector.tensor_scalar(
                        out=res[:, i * wid : (i + 1) * wid],
                        in0=pss[t][:, i * wid : (i + 1) * wid],
                        scalar1=DRAIN_SCALE[pl],
                        scalar2=None,
                        op0=mybir.AluOpType.mult,
                    )
                off = 2 * W * t
                step = w_all // halves
                for h in range(halves):
                    o0, o1 = h * step, (h + 1) * step if h < halves - 1 else w_all
                    eng = nc.sync if (halves == 2 and h == 0) else nc.scalar
                    eng.dma_start(
                        out=o_d[:, off + o0 : off + o1], in_=res[:, o0:o1]
                    )

            # phase A: chunk-major (all 3 banks per chunk) while DMA ramps
            for gi in range(PHASE_A_GROUPS):
                k0, gn = KGROUPS[gi]
                t_sb = unpack(list(range(PLANES)), k0, gn, gi)
                for kk in range(k0, k0 + gn):
                    for t in range(3):
                        bank_mm(t, kk, t_sb, 2 * t, k0)
            # phase B: bank-major; drain(t) hides under bank t+1's matmuls
            bgroups = [
                (k0, gn) for k0, gn in KGROUPS[PHASE_A_GROUPS:]
            ]
            for t in range(3):
                for bi, (k0, gn) in enumerate(bgroups):
                    t_sb = unpack(list(BANKPL[t]), k0, gn, f"b{t}_{bi}")
                    for kk in range(k0, k0 + gn):
                        bank_mm(t, kk, t_sb, 0, k0)
                # patch chunk closes bank t's accumulation
                npl = len(BANKPL[t])
                prhs = rmat_sb[:, 2 * t : 2 * t + npl, : BANKW[t] // npl]
                nc.tensor.matmul(
                    pss[t][:], xp_sb[:], prhs, start=False, stop=True
                )
                drain(t, halves=2 if t == 2 else 1)

    nc.compile()
    return nc


def _prepare(x: np.ndarray, edge_index: np.ndarray):
    ei = np.asarray(edge_index)
    src = ei[0].astype(np.int64)
    dst = ei[1].astype(np.int64)

    xf = np.asarray(x).astype(np.float32)
    xp = np.zeros((NPAD, D_FEAT), np.float16)
    xp[:N_NODES] = xf
    # xt[p, k, :] = x[k*128 + p, :]
    xt = np.ascontiguousarray(
        xp.reshape(KCH, P, D_FEAT).transpose(1, 0, 2).reshape(P, KCH * D_FEAT)
    )

    shifts = (1 << np.arange(PLANES, dtype=np.uint32))[None, :, None]
    bitvals = np.array(BITVAL, np.float32)

    in_maps = []
    for c in range(N_CORES):
        sel = (dst >= c * DCORE) & (dst < (c + 1) * DCORE)
        idx = src[sel] * DCORE + (dst[sel] - c * DCORE)
        cnt = np.bincount(idx, minlength=NPAD * DCORE).reshape(NPAD, DCORE)
        base = np.minimum(cnt, 1)

        g = np.zeros((NPAD, PLANES, W), np.uint32)
        g.reshape(NPAD, PLANES * W)[:, :DCORE] = base
        byte = (g * shifts).sum(axis=1).astype(np.uint8)  # [NPAD, W]
        apk = np.ascontiguousarray(
            byte.reshape(KCH, P, W).transpose(1, 0, 2).reshape(P, KCH * W)
        )

        resid = (cnt - base).astype(np.int64)
        rs, cs = np.nonzero(resid)
        uniq = np.unique(rs)
        assert len(uniq) <= P, f"core {c}: {len(uniq)} patch rows > {P}"
        assert resid.max(initial=0) <= 15
        slot_of = np.zeros(NPAD, np.int64)
        slot_of[uniq] = np.arange(len(uniq))
        pidx = np.zeros((P, 1), np.int32)
        pidx[: len(uniq), 0] = uniq.astype(np.int32)
        rmat = np.zeros((P, PLANES * W), np.float32)
        if len(rs):
            pl = cs // W
            j = cs % W
            rmat[slot_of[rs], pl * W + j] = resid[rs, cs] * bitvals[pl]
        in_maps.append(
            {
                "xt": xt,
                "apk": apk,
                "xr": xp,
                "pidx": pidx,
                "rmat": rmat.astype(FP8),
            }
        )
    return in_maps


def kernel(x: np.ndarray, edge_index: np.ndarray) -> np.ndarray:
    global LAST_RESULT
    in_maps = _prepare(x, edge_index)
    nc = _build_program()
    res = run_bass_kernel_spmd(
        nc,
        in_maps,
        list(range(N_CORES)),
        trace=TRACE,
        trace_cores=TRACE_CORES if TRACE else None,
    )
    LAST_RESULT = res
    # o per core: [128 f, DCORE d] fp16 -> out[c*DCORE + d, f] f32
    out = np.concatenate(
        [np.asarray(r["o"]).astype(np.float32).T for r in res.results], axis=0
    )
    return np.ascontiguousarray(out[:N_NODES])


if __name__ == "__main__":
    rng = np.random.default_rng(0)
    x = rng.standard_normal((N_NODES, D_FEAT), dtype=np.float32)
    edge_index = rng.integers(0, N_NODES, size=(2, 320000)).astype(np.int64)
    out = kernel(x, edge_index)
    ref = np.zeros((N_NODES, D_FEAT), np.float32)
    np.add.at(ref, edge_index[1], x[edge_index[0]])
    rel = np.linalg.norm(out - ref) / np.linalg.norm(ref)
    print("rel L2 err:", rel)


# revision 11
# speedup vs baseline: 1.0349x; 1.0349x over previous
"""GNN message-passing (scatter-add) kernel for 8 Trainium2 NeuronCores.

Computes out = segment_sum(x[src], dst, num_segments=N) for
x [10000, 128] f32, edge_index [2, 320000] int64.

Strategy — BIT-PLANE dense count-matrix matmul (no per-edge work):
  out^T[f, d] = sum_s A[s, d] * x[s, f]   with A[s, d] = #edges s->d.

  A is ~0/1 (density 0.3%). Store min(A, 1) as 5 bit-planes packed one
  byte per 5 dst columns (bit pl of byte[s, j] = A[s, 256*pl + j] >= 1):
  the A stream shrinks from 12.8MB dense fp8 to 2.6MB per core.

  On device the DVE expands each plane with one bitwise-AND
  (mask 0x0101<<pl on a uint16 view, two fp8 lanes per elem). The
  masked bytes ARE valid fp8e4m3: a lone bit pl is the exact power of
  two v_pl in [2^-9 .. 2^-5], so the matmul accumulates v_pl * partial
  and the PSUM drain rescales by the exact inverse.

  Cells with count >= 2 (~60 per core) are patched by one extra
  contraction chunk: their x rows are fetched on-device by an indirect
  (indexed) DMA, and a host-built residual matrix rmat (= (count-1) *
  v_pl at the patched cells) joins the same PSUM accumulation.

  dst is sharded across the 8 cores (core c owns cols [c*1264, +1264));
  no collectives. x rides fp16; out written fp16 (rel err ~3e-4).

  Schedule: matmuls start chunk-major while the DMA stream ramps
  (3 psum banks fed per chunk), then switch bank-major so each PSUM
  bank's drain + store overlaps the next bank's matmuls; only the last
  bank's drain is exposed. LDWEIGHTS fully overlaps the matmul stream
  (verified on traces), so the kernel runs at the PE column rate:
  79*1264 fp8 columns ~= 43us, plus ~5us preamble and ~3us tail.
"""

import sys

for _p in ("/opt/trn_rl_repo",):
    if _p not in sys.path:
        sys.path.append(_p)


def _install_axon_ntff_hook_shim():
    # Some images ship an antenv without axon_hooks; bass_utils then
    # crashes on trace=True under axon. Provide the module and register
    # the ctypes NTFF hook the same way trn_boot would. Fully guarded —
    # a no-op wherever the real module exists.
    import types

    try:
        import antenv.axon_hooks  # noqa: F401

        return
    except ImportError:
        pass
    try:
        import antenv

        mod = types.ModuleType("antenv.axon_hooks")
        mod._hook = None

        def set_axon_ntff_profile_hook(h):
            mod._hook = h

        def get_axon_ntff_profile_hook():
            return mod._hook

        mod.set_axon_ntff_profile_hook = set_axon_ntff_profile_hook
        mod.get_axon_ntff_profile_hook = get_axon_ntff_profile_hook
        sys.modules["antenv.axon_hooks"] = mod
        antenv.axon_hooks = mod
        from trn_agent_boot.trn_boot import _ntff_profile_via_ctypes

        mod._hook = _ntff_profile_via_ctypes("/opt/axon/libaxon_pjrt.so")
    except Exception:
        pass


_install_axon_ntff_hook_shim()

import ml_dtypes
import numpy as np

import concourse.bacc as bacc
import concourse.mybir as mybir
import concourse.tile as tile
from concourse import bass
from concourse.bass_utils import run_bass_kernel_spmd

N_NODES = 10000
D_FEAT = 128
N_CORES = 8
P = 128
KCH = -(-N_NODES // P)  # 79 source chunks
NPAD = KCH * P  # 10112
DCORE = NPAD // N_CORES  # 1264 dst columns per core
PLANES = 5
W = 256  # dst cols per plane; 5*256 = 1280 >= 1264 (last plane 240 used)
WU = W // 2  # uint16 lanes per plane
W_LAST = DCORE - 4 * W  # 240
# fp8e4m3 value of a lone bit pl (exact powers of two)
BITVAL = [2.0**-9, 2.0**-8, 2.0**-7, 2.0**-6, 2.0**-5]
DRAIN_SCALE = [512.0, 256.0, 128.0, 64.0, 32.0]
# psum bank t covers planes BANKPL[t] with total width BANKW[t]
BANKPL = [(0, 1), (2, 3), (4,)]
BANKW = [2 * W, 2 * W, W_LAST]
FP8 = ml_dtypes.float8_e4m3

# test/profiling hooks
TRACE = False
TRACE_CORES = None
LAST_RESULT = None


def _groups(sizes):
    out = []
    k0 = 0
    for g in sizes:
        out.append((k0, g))
        k0 += g
    assert k0 == KCH
    return out


# chunk-load groups; the first ones are small for a fast PE start
GSIZES = [2, 2, 4, 8, 8, 8, 8, 8, 8, 8, 8, 7]
KGROUPS = _groups(GSIZES)
# chunks 0..KSTAG-1 run chunk-major (all banks per chunk, matching the
# DMA stream rate); the last chunks run bank-major so bank 0/1 drains
# overlap bank 1/2 matmuls
KSTAG = KCH - 8


def _build_program():
    nc = bacc.Bacc(
        "TRN2", target_bir_lowering=False, debug=False, num_devices=N_CORES
    )
    xt_d = nc.dram_tensor(
        "xt", [P, KCH * D_FEAT], mybir.dt.float16, kind="ExternalInput"
    )
    apk_d = nc.dram_tensor(
        "apk", [P, KCH * W], mybir.dt.uint8, kind="ExternalInput"
    )
    xr_d = nc.dram_tensor(
        "xr", [NPAD, D_FEAT], mybir.dt.float16, kind="ExternalInput"
    )
    pidx_d = nc.dram_tensor("pidx", [P, 1], mybir.dt.int32, kind="ExternalInput")
    rmat_d = nc.dram_tensor(
        "rmat", [P, PLANES * W], mybir.dt.float8e4, kind="ExternalInput"
    )
    o_d = nc.dram_tensor("o", [P, DCORE], mybir.dt.float16, kind="ExternalOutput")

    xv = xt_d[:].rearrange("p (k f) -> p k f", k=KCH, f=D_FEAT)
    av = apk_d[:].rearrange("p (k w) -> p k w", k=KCH, w=W)

    with tile.TileContext(nc) as tc:
        with (
            tc.tile_pool(name="xt", bufs=1) as xtp,
            tc.tile_pool(name="a", bufs=1) as ap_,
            tc.tile_pool(name="pl", bufs=2) as plp,
            tc.tile_pool(name="patch", bufs=1) as pp,
            tc.tile_pool(name="res", bufs=2) as resp,
            tc.tile_pool(name="ps", bufs=1, space="PSUM") as psp,
        ):
            # xt and apk stay fully resident; loads stream group-wise.
            # The first groups ride the gpsimd queue, which comes up
            # earlier than sync/scalar; later groups split across the
            # sync (apk) and scalar (xt) HWDGE queues. One persistent
            # tile PER GROUP so deps are group-granular regardless of
            # the framework's AP tracking.
            xt_g = {}
            a16_g = {}
            for gi, (k0, gn) in enumerate(KGROUPS):
                xg = xtp.tile(
                    [P, gn, D_FEAT], mybir.dt.float16,
                    tag=f"xt{gi}", name=f"xt{gi}",
                )
                ag = ap_.tile(
                    [P, gn, W], mybir.dt.uint8, tag=f"a{gi}", name=f"a{gi}"
                )
                a_eng = nc.gpsimd if gi <= 1 else nc.sync
                x_eng = nc.gpsimd if gi <= 1 else nc.scalar
                a_eng.dma_start(out=ag[:], in_=av[:, k0 : k0 + gn, :])
                x_eng.dma_start(out=xg[:], in_=xv[:, k0 : k0 + gn, :])
                xt_g[k0] = xg
                a16_g[k0] = ag[:].bitcast(mybir.dt.uint16)  # [P, gn, WU]

            # patch inputs ride the gpsimd queue after the first groups
            pidx_sb = pp.tile([P, 1], mybir.dt.int32, name="pidx_sb")
            nc.gpsimd.dma_start(out=pidx_sb[:], in_=pidx_d[:])
            rmat_sb = pp.tile([P, PLANES, W], mybir.dt.float8e4, name="rmat_sb")
            nc.gpsimd.dma_start(
                out=rmat_sb[:],
                in_=rmat_d[:].rearrange("p (l w) -> p l w", l=PLANES, w=W),
            )
            xp_sb = pp.tile([P, D_FEAT], mybir.dt.float16, name="xp_sb")
            nc.gpsimd.indirect_dma_start(
                out=xp_sb[:],
                out_offset=None,
                in_=xr_d[:],
                in_offset=bass.IndirectOffsetOnAxis(ap=pidx_sb[:, :1], axis=0),
            )

            pss = [
                psp.tile(
                    [P, BANKW[t]], mybir.dt.float32, tag=f"ps{t}", name=f"ps{t}"
                )
                for t in range(3)
            ]

            def unpack(pls, k0, gn, gi):
                # one AND per plane over a chunk group; masked bytes are
                # read back as fp8 by the matmuls
                t_sb = plp.tile(
                    [P, gn, len(pls), WU],
                    mybir.dt.uint16,
                    tag=f"pl{len(pls)}_{gn}",
                    name=f"pl{pls[0]}_{gi}",
                )
                for i, pl in enumerate(pls):
                    nc.vector.tensor_scalar(
                        out=t_sb[:, :, i, :],
                        in0=a16_g[k0],
                        scalar1=(0x0101 << pl) & 0xFFFF,
                        scalar2=None,
                        op0=mybir.AluOpType.bitwise_and,
                    )
                return t_sb

            def bank_mm(t, kk, t_sb, i0, g0):
                # matmul for psum bank t, contraction chunk kk; planes of
                # bank t sit at index i0.. in t_sb (group base g0)
                npl = len(BANKPL[t])
                wid = BANKW[t] // npl // 2  # u16 lanes per plane used
                rhs = t_sb[:, kk - g0, i0 : i0 + npl, :wid].bitcast(
                    mybir.dt.float8e4
                )
                nc.tensor.matmul(
                    pss[t][:],
                    xt_g[g0][:, kk - g0, :],
                    rhs,
                    start=(kk == 0),
                    stop=False,
                )

            def drain(t, halves=1):
                w_all = BANKW[t]
                res = resp.tile(
                    [P, w_all], mybir.dt.float16, tag=f"res{t}", name=f"res{t}"
                )
                for i, pl in enumerate(BANKPL[t]):
                    wid = w_all // len(BANKPL[t])
                    nc.vector.tensor_scalar(
                        out=res[:, i * wid : (i + 1) * wid],
                        in0=pss[t][:, i * wid : (i + 1) * wid],
                        scalar1=DRAIN_SCALE[pl],
                        scalar2=None,
                        op0=mybir.AluOpType.mult,
                    )
                off = 2 * W * t
                step = w_all // halves
                for h in range(halves):
                    o0, o1 = h * step, (h + 1) * step if h < halves - 1 else w_all
                    eng = nc.sync if (halves == 2 and h == 0) else nc.scalar
                    eng.dma_start(
                        out=o_d[:, off + o0 : off + o1], in_=res[:, o0:o1]
                    )

            # main sweep: chunk-major (all 3 banks per chunk, matching
            # the just-in-time DMA stream) up to KSTAG
            tiles = {}  # k0 -> plane tile
            for gi, (k0, gn) in enumerate(KGROUPS):
                tiles[k0] = unpack(list(range(PLANES)), k0, gn, gi)
                for kk in range(k0, min(k0 + gn, KSTAG)):
                    for t in range(3):
                        bank_mm(t, kk, tiles[k0], 2 * t, k0)

            def group_of(kk):
                for k0, gn in KGROUPS:
                    if k0 <= kk < k0 + gn:
                        return k0
                raise AssertionError(kk)

            # tail stagger: finish banks one at a time so drain(t)
            # hides under bank t+1's matmuls; data is long resident
            for t in range(3):
                for kk in range(KSTAG, KCH):
                    k0 = group_of(kk)
                    bank_mm(t, kk, tiles[k0], 2 * t, k0)
                # patch chunk closes bank t's accumulation
                npl = len(BANKPL[t])
                prhs = rmat_sb[:, 2 * t : 2 * t + npl, : BANKW[t] // npl]
                nc.tensor.matmul(
                    pss[t][:], xp_sb[:], prhs, start=False, stop=True
                )
                drain(t, halves=2 if t == 2 else 1)

    nc.compile()
    return nc


def _prepare(x: np.ndarray, edge_index: np.ndarray):
    ei = np.asarray(edge_index)
    src = ei[0].astype(np.int64)
    dst = ei[1].astype(np.int64)

    xf = np.asarray(x).astype(np.float32)
    xp = np.zeros((NPAD, D_FEAT), np.float16)
    xp[:N_NODES] = xf
    # xt[p, k, :] = x[k*128 + p, :]
    xt = np.ascontiguousarray(
        xp.reshape(KCH, P, D_FEAT).transpose(1, 0, 2).reshape(P, KCH * D_FEAT)
    )

    shifts = (1 << np.arange(PLANES, dtype=np.uint32))[None, :, None]
    bitvals = np.array(BITVAL, np.float32)

    in_maps = []
    for c in range(N_CORES):
        sel = (dst >= c * DCORE) & (dst < (c + 1) * DCORE)
        idx = src[sel] * DCORE + (dst[sel] - c * DCORE)
        cnt = np.bincount(idx, minlength=NPAD * DCORE).reshape(NPAD, DCORE)
        base = np.minimum(cnt, 1)

        g = np.zeros((NPAD, PLANES, W), np.uint32)
        g.reshape(NPAD, PLANES * W)[:, :DCORE] = base
        byte = (g * shifts).sum(axis=1).astype(np.uint8)  # [NPAD, W]
        apk = np.ascontiguousarray(
            byte.reshape(KCH, P, W).transpose(1, 0, 2).reshape(P, KCH * W)
        )

        resid = (cnt - base).astype(np.int64)
        rs, cs = np.nonzero(resid)
        uniq = np.unique(rs)
        assert len(uniq) <= P, f"core {c}: {len(uniq)} patch rows > {P}"
        assert resid.max(initial=0) <= 15
        slot_of = np.zeros(NPAD, np.int64)
        slot_of[uniq] = np.arange(len(uniq))
        pidx = np.zeros((P, 1), np.int32)
        pidx[: len(uniq), 0] = uniq.astype(np.int32)
        rmat = np.zeros((P, PLANES * W), np.float32)
        if len(rs):
            pl = cs // W
            j = cs % W
            rmat[slot_of[rs], pl * W + j] = resid[rs, cs] * bitvals[pl]
        in_maps.append(
            {
                "xt": xt,
                "apk": apk,
                "xr": xp,
                "pidx": pidx,
                "rmat": rmat.astype(FP8),
            }
        )
    return in_maps


def kernel(x: np.ndarray, edge_index: np.ndarray) -> np.ndarray:
    global LAST_RESULT
    in_maps = _prepare(x, edge_index)
    nc = _build_program()
    res = run_bass_kernel_spmd(
        nc,
        in_maps,
        list(range(N_CORES)),
        trace=TRACE,
        trace_cores=TRACE_CORES if TRACE else None,
    )
    LAST_RESULT = res
    # o per core: [128 f, DCORE d] fp16 -> out[c*DCORE + d, f] f32
    out = np.concatenate(
        [np.asarray(r["o"]).astype(np.float32).T for r in res.results], axis=0
    )
    return np.ascontiguousarray(out[:N_NODES])


if __name__ == "__main__":
    rng = np.random.default_rng(0)
    x = rng.standard_normal((N_NODES, D_FEAT), dtype=np.float32)
    edge_index = rng.integers(0, N_NODES, size=(2, 320000)).astype(np.int64)
    out = kernel(x, edge_index)
    ref = np.zeros((N_NODES, D_FEAT), np.float32)
    np.add.at(ref, edge_index[1], x[edge_index[0]])
    rel = np.linalg.norm(out - ref) / np.linalg.norm(ref)
    print("rel L2 err:", rel)
